# revision 1
# baseline (speedup 1.0000x reference)
"""Trainium2 Bass kernel for nn_CellAnnotator (per-pixel 8x8 locally-connected
weighted pooling with normalization), SPMD across 8 NeuronCores.

Contract: kernel(**inputs) takes FULL inputs (x0 [512,512,128] f32,
weights [512,512,64] f32, cnts [512,512,1] f32) and returns the FULL
output [512,512,128] f32.

Sharding: rows (H) split across 8 cores, 64 output rows each; input shards
carry the 3+4-row halo host-side (zero-padded at borders) -> no d2d comms.

Algorithm (oct-row banded matmul):
  Output rows are processed in groups of S=8 (8 rowgroups/core); pixels in
  57 blocks of 9 (input window = 16 cols).  One matmul per (rowgroup,
  block, oct): lhsT = banded weights [K=128 (8 rows x 16 cols), M=72
  (8 rows x 9 px)], rhs = x oct tile [128, 128ch]; 2 octs accumulate in
  PSUM.  cnts pooling runs as narrow N=1 matmuls into a shared [72,57]
  PSUM tile, so normalization is one add+reciprocal per rowgroup.

  Traffic per core: x octs 16.9MB (no row duplication, 9 tiles), banded
  weights 16.8MB (4x inflation vs compact -- the best canonical-AP
  banding), output 8.6MB; 25 DMAs total.
"""

import numpy as np
import ml_dtypes
from contextlib import ExitStack

import concourse.bass as bass
import concourse.bacc as bacc
import concourse.mybir as mybir
import concourse.tile as tile
from concourse.ap import AP
from concourse.bass_utils import run_bass_kernel_spmd

BF16 = np.dtype(ml_dtypes.bfloat16)

# Problem constants (hardcoded per contract)
H, W, C = 512, 512, 128
ROI = 8
NCORES = 8
ROWS = H // NCORES             # 64 output rows per core
CCH = C + 1                    # x channels + cnts as channel 128

S = 8                          # output rows per rowgroup
RG = ROWS // S                 # 8 rowgroups per core
BLK = 9                        # output pixels per column block
NB = 57                        # column blocks (57*9 = 513 >= 512)
M = S * BLK                    # 72 matmul output partitions
NQ = 9                         # oct tiles per core (input rows 0..71)
NT = 2                         # octs (accumulation steps) per rowgroup
LINE = NT * NB                 # 114 elems per (s,jj) line
FREESZ = S * BLK * LINE        # 8208 elems per band partition
XFREE = NB * CCH               # 7353 elems per x oct partition

_CACHE = {}


def _build_nc(rep=1, variant="full"):
    f32 = mybir.dt.float32
    bf = mybir.dt.bfloat16
    nc = bacc.Bacc("TRN2", target_bir_lowering=False, debug=False,
                   num_devices=NCORES)
    xq = nc.dram_tensor("xq", [NQ, 128, XFREE], bf, kind="ExternalInput")
    bnd = nc.dram_tensor("bnd", [RG, 128, FREESZ], bf, kind="ExternalInput")
    outb = nc.dram_tensor("outb", [RG, M, NB * C], bf, kind="ExternalOutput")

    with tile.TileContext(nc) as tc:
        with ExitStack() as ctx:
            if rep > 1:
                ctx.enter_context(tc.For_i(0, rep, 1))
            xpool = ctx.enter_context(tc.tile_pool(name="xp", bufs=4))
            bpool = ctx.enter_context(tc.tile_pool(name="bp", bufs=3))
            ppool = ctx.enter_context(
                tc.tile_pool(name="pp", bufs=6, space="PSUM"))
            cpool = ctx.enter_context(
                tc.tile_pool(name="cp", bufs=2, space="PSUM"))
            opool = ctx.enter_context(tc.tile_pool(name="op", bufs=3))
            spool = ctx.enter_context(tc.tile_pool(name="sp", bufs=4))

            xcache = {}

            HX = (NB // 2 + 1) * CCH

            def get_x(k):
                if k not in xcache:
                    # halves: block-b matmuls only read [b*CCH:...], so the
                    # first half-pass can start while half 2 is in flight
                    t = xpool.tile([128, XFREE], bf, tag="xt")
                    nc.sync.dma_start(
                        t[:, 0:HX], AP(xq, k * 128 * XFREE,
                                       [[XFREE, 128], [1, HX]]))
                    nc.sync.dma_start(
                        t[:, HX:], AP(xq, k * 128 * XFREE + HX,
                                      [[XFREE, 128], [1, XFREE - HX]]))
                    xcache[k] = t
                return xcache[k]

            for R in range(RG):
                bt = bpool.tile([128, S, BLK, LINE], bf, tag="bt")
                nc.gpsimd.dma_start(
                    bt[:], AP(bnd, R * 128 * FREESZ,
                              [[FREESZ, 128], [1, FREESZ]]))
                if variant == "dmab":
                    continue
                xts = [get_x(R + t) for t in range(NT)]
                if variant == "dma":
                    continue

                # two half-passes per rowgroup: cnts pooling + normalize +
                # store for blocks [0,29) depend only on the first halves of
                # the oct tiles, so they overlap the second-half loads
                psc = cpool.tile([M, NB], f32, tag="psc")
                rec = spool.tile([M, NB], f32, tag="rec")
                ot = opool.tile([M, NB * C], bf, tag="ot")
                NBH = NB // 2 + 1
                for b0, b1 in ((0, NBH), (NBH, NB)):
                    for b in range(b0, b1):
                        for t in range(NT):
                            nc.tensor.matmul(
                                psc[:, b:b + 1], bt[:, :, :, t * NB + b],
                                AP(xts[t][:].tensor, b * CCH + C,
                                   [[XFREE, 128], [1, 1]]),
                                start=(t == 0), stop=(t == NT - 1))
                    nc.vector.tensor_scalar_add(
                        rec[:, b0:b1], psc[:, b0:b1], 1e-6)
                    nc.vector.reciprocal(rec[:, b0:b1], rec[:, b0:b1])
                    for b in range(b0, b1):
                        psum = ppool.tile([M, C], f32, tag="ps")
                        for t in range(NT):
                            nc.tensor.matmul(
                                psum[:], bt[:, :, :, t * NB + b],
                                AP(xts[t][:].tensor, b * CCH,
                                   [[XFREE, 128], [1, C]]),
                                start=(t == 0), stop=(t == NT - 1))
                        if variant == "pe":
                            continue
                        odst = ot[:, b * C:(b + 1) * C]
                        # NOTE: GPSIMD cannot access PSUM on HW (sim allows)
                        if b % 2 == 0:
                            nc.vector.tensor_scalar(
                                odst, psum[:], rec[:, b:b + 1], None,
                                op0=mybir.AluOpType.mult)
                        else:
                            nc.scalar.activation(
                                odst, psum[:],
                                mybir.ActivationFunctionType.Identity,
                                scale=rec[:, b:b + 1])
                    if variant == "pe":
                        continue
                    nc.scalar.dma_start(
                        AP(outb, R * M * NB * C + b0 * C,
                           [[NB * C, M], [1, (b1 - b0) * C]]),
                        ot[:, b0 * C:b1 * C])
    nc.compile()
    return nc


def _get_nc(rep=1, variant="full"):
    key = ("nc", rep, variant)
    if key not in _CACHE:
        _CACHE[key] = _build_nc(rep, variant)
    return _CACHE[key]


def _host_prep(x0, weights, cnts):
    # Padded global image: row g -> Xpad[g+3], col c -> +3, 129 channels.
    Xpad = np.zeros((H + 8, W + 21, CCH), BF16)
    Xpad[3:3 + H, 3:3 + W, :C] = x0.astype(BF16)
    Xpad[3:3 + H, 3:3 + W, C] = cnts[:, :, 0].astype(BF16)

    wq = weights.reshape(H, W, ROI, ROI)
    wpad = np.zeros((H, W + 13, ROI, ROI), np.float32)
    wpad[:, :W] = wq

    # band index grids: [R, rt, u, s, jj, t, b]
    Rg = np.arange(RG).reshape(-1, 1, 1, 1, 1, 1, 1)
    rt = np.arange(S).reshape(1, -1, 1, 1, 1, 1, 1)
    ug = np.arange(16).reshape(1, 1, -1, 1, 1, 1, 1)
    sg = np.arange(S).reshape(1, 1, 1, -1, 1, 1, 1)
    jj = np.arange(BLK).reshape(1, 1, 1, 1, -1, 1, 1)
    tg = np.arange(NT).reshape(1, 1, 1, 1, 1, -1, 1)
    bg = np.arange(NB).reshape(1, 1, 1, 1, 1, 1, -1)
    p = 8 * tg + rt - sg
    q = ug - jj
    cols = BLK * bg + jj
    mask = (p >= 0) & (p < ROI) & (q >= 0) & (q < ROI) & (cols < W)
    pc = np.clip(p, 0, ROI - 1)
    qc = np.clip(q, 0, ROI - 1)

    in_maps = []
    sr, sc, sch = Xpad.strides
    for k in range(NCORES):
        R0 = k * ROWS
        rows = R0 + S * Rg + sg
        val = wpad[rows, cols, pc, qc] * mask
        # val dims [R, rt, u, s, jj, t, b] -> [R, 128=(rt,u), (s,jj,t,b)]
        bndarr = np.ascontiguousarray(
            val.reshape(RG, 128, S, BLK, LINE).reshape(
                RG, 128, FREESZ).astype(BF16))
        xbase = Xpad[R0:R0 + 76]
        xv = np.lib.stride_tricks.as_strided(
            xbase, shape=(NQ, S, 16, NB, CCH),
            strides=(S * sr, sr, sc, BLK * sc, sch))
        xqarr = np.ascontiguousarray(xv.reshape(NQ, 128, XFREE))
        in_maps.append({"xq": xqarr, "bnd": bndarr})
    return in_maps


def _unpack(outb):
    """outb [RG, 72, 57*128] -> [64, 512, 128] f32."""
    o = outb.astype(np.float32).reshape(RG, S, BLK, NB, C)
    o = o.transpose(0, 1, 3, 2, 4).reshape(ROWS, NB * BLK, C)
    return o[:, :W]


def kernel(x0, weights, cnts):
    x0 = np.asarray(x0, np.float32)
    weights = np.asarray(weights, np.float32)
    cnts = np.asarray(cnts, np.float32)
    nc = _get_nc()
    in_maps = _host_prep(x0, weights, cnts)
    res = run_bass_kernel_spmd(nc, in_maps, core_ids=list(range(NCORES)))
    return np.ascontiguousarray(np.concatenate(
        [_unpack(res.results[k]["outb"]) for k in range(NCORES)], axis=0))



# revision 4
# speedup vs baseline: 1.7427x; 1.7427x over previous
"""Trainium2 Bass kernel for nn_CellAnnotator (per-pixel 8x8 locally-connected
weighted pooling with normalization), SPMD across 8 NeuronCores.

Contract: kernel(**inputs) takes FULL inputs (x0 [512,512,128] f32,
weights [512,512,64] f32, cnts [512,512,1] f32) and returns the FULL
output [512,512,128] f32.

Sharding: rows (H) split across 8 cores, 64 output rows each; input shards
carry the 3+4-row halo host-side (zero-padded at borders) -> no d2d comms.

Algorithm (oct-row banded matmul, v2):
  Output rows in groups of S=8 (8 rowgroups/core); pixels in 57 blocks of 9
  (input window = 16 cols).  cnts ride as channel 128 of the x tiles, so ONE
  matmul per (rowgroup, block, oct) computes both pooled x and pooled cnts:
  lhsT = banded weights [K=128 (8 rows x 16 cols), M=72 (8 rows x 9 px)],
  rhs = x oct tile slice [128, 129]; 2 octs accumulate in PSUM.  Blocks are
  packed 3 per PSUM bank ([72, 387] f32), so the cnt columns of a chunk sit
  at stride 129 and one strided DVE reciprocal serves 3 blocks; the
  normalizing multiply alternates vector/scalar engines.

  Band tiles are stored zero-skip: layout [128, t, s, jj, b]; for column
  (s,jj) the oct-t band is nonzero only at partitions rt>=s (t=0) / rt<s
  (t=1), so HBM holds just those slabs (2.1x inflation instead of 4.2x) and
  the structural zeros are memset once into two fixed SBUF buffers.

  Traffic per core: x octs 16.9MB, banded weights 8.4MB, output 8.6MB.
"""

import numpy as np
import ml_dtypes
from contextlib import ExitStack

import concourse.bass as bass
import concourse.bacc as bacc
import concourse.mybir as mybir
import concourse.tile as tile
from concourse.ap import AP
from concourse.bass_utils import run_bass_kernel_spmd

BF16 = np.dtype(ml_dtypes.bfloat16)

# Problem constants (hardcoded per contract)
H, W, C = 512, 512, 128
ROI = 8
NCORES = 8
ROWS = H // NCORES             # 64 output rows per core
CCH = C + 1                    # x channels + cnts as channel 128

S = 8                          # output rows per rowgroup
RG = ROWS // S                 # 8 rowgroups per core
BLK = 9                        # output pixels per column block
NB = 57                        # column blocks (57*9 = 513 >= 512)
M = S * BLK                    # 72 matmul output partitions
NQ = 9                         # oct tiles per core (input rows 0..71)
NT = 2                         # octs (accumulation steps) per rowgroup
CH = 3                         # blocks per PSUM bank chunk (3*129 <= 512 f32)
NCH = NB // CH                 # 19 chunks per rowgroup
XFREE = NB * CCH               # 7353 elems per x oct partition

# zero-skip band slabs: per rowgroup, order (t=0, s=0..7), (t=1, s=1..7);
# slab (t=0,s): partitions [16s,128); (t=1,s): partitions [0,16s).
# free extent per (t,s,partition): 9*57 = 513 elems.
_SLABS = []          # (t, s, part0, nparts, elem_offset)
_off = 0
for _t in range(NT):
    for _s in range(S):
        _p0, _np_ = (16 * _s, 128 - 16 * _s) if _t == 0 else (0, 16 * _s)
        if _np_ == 0:
            continue
        _SLABS.append((_t, _s, _p0, _np_, _off))
        _off += _np_ * BLK * NB
BNDSZ = _off         # 525312 elems per rowgroup

_CACHE = {}


def _build_nc(rep=1, variant="full"):
    f32 = mybir.dt.float32
    bf = mybir.dt.bfloat16
    nc = bacc.Bacc("TRN2", target_bir_lowering=False, debug=False,
                   num_devices=NCORES)
    xq = nc.dram_tensor("xq", [NQ, 128, XFREE], bf, kind="ExternalInput")
    bnd = nc.dram_tensor("bnd", [RG, BNDSZ], bf, kind="ExternalInput")
    outb = nc.dram_tensor("outb", [RG, M, NB * C], bf, kind="ExternalOutput")

    with tile.TileContext(nc) as tc:
        with ExitStack() as ctx:
            xpool = ctx.enter_context(tc.tile_pool(name="xp", bufs=4))
            bpool = ctx.enter_context(tc.tile_pool(name="bp", bufs=2))
            ppool = ctx.enter_context(
                tc.tile_pool(name="pp", bufs=6, space="PSUM"))
            opool = ctx.enter_context(tc.tile_pool(name="op", bufs=3))
            spool = ctx.enter_context(tc.tile_pool(name="sp", bufs=3))

            # two fixed band buffers, structural zeros established once
            # (outside the rep loop); per-R DMAs only overwrite the
            # nonzero slabs, which are identical regions every iteration.
            bts = [bpool.tile([128, NT, S, BLK, NB], bf, tag="bt",
                              name=f"bt{i}")
                   for i in range(2)]
            nc.vector.memset(bts[0][:], 0.0)
            nc.gpsimd.memset(bts[1][:], 0.0)

            if rep > 1:
                ctx.enter_context(tc.For_i(0, rep, 1))

            xcache = {}
            HX = (NB // 2 + 1) * CCH

            def get_x(k):
                if k not in xcache:
                    # halves: block-b matmuls only read [b*CCH:...], so the
                    # first half-pass can start while half 2 is in flight
                    t = xpool.tile([128, XFREE], bf, tag="xt")
                    nc.sync.dma_start(
                        t[:, 0:HX], AP(xq, k * 128 * XFREE,
                                       [[XFREE, 128], [1, HX]]))
                    nc.sync.dma_start(
                        t[:, HX:], AP(xq, k * 128 * XFREE + HX,
                                      [[XFREE, 128], [1, XFREE - HX]]))
                    xcache[k] = t
                return xcache[k]

            for R in range(RG):
                bt = bts[R % 2]
                for (t, s, p0, nparts, off) in _SLABS:
                    nc.gpsimd.dma_start(
                        bt[p0:p0 + nparts, t, s, :, :],
                        AP(bnd, R * BNDSZ + off,
                           [[BLK * NB, nparts], [1, BLK * NB]]))
                if variant == "dmab":
                    continue
                xts = [get_x(R + t) for t in range(NT)]
                if variant == "dma":
                    continue

                rec = spool.tile([M, NB], f32, tag="rec")
                ot = opool.tile([M, NB * C], bf, tag="ot")
                NBH = NB // 2 + 1
                for c in range(NCH):
                    psum = ppool.tile([M, CH * CCH], f32, tag="ps")
                    for j in range(CH):
                        b = c * CH + j
                        for t in range(NT):
                            nc.tensor.matmul(
                                psum[:, j * CCH:(j + 1) * CCH],
                                bt[:, t, :, :, b],
                                AP(xts[t][:].tensor, b * CCH,
                                   [[XFREE, 128], [1, CCH]]),
                                start=(t == 0), stop=(t == NT - 1))
                    if variant == "pe":
                        continue
                    # batched reciprocal of the 3 cnt columns (stride 129)
                    nc.vector.reciprocal(
                        rec[:, c * CH:(c + 1) * CH],
                        AP(psum[:].tensor, C, [[CH * CCH, M], [CCH, CH]]))
                    for j in range(CH):
                        b = c * CH + j
                        odst = ot[:, b * C:(b + 1) * C]
                        psrc = AP(psum[:].tensor, j * CCH,
                                  [[CH * CCH, M], [1, C]])
                        rsc = rec[:, c * CH + j:c * CH + j + 1]
                        # NOTE: GPSIMD cannot access PSUM on HW (sim allows)
                        if b % 2 == 0:
                            nc.vector.tensor_scalar(
                                odst, psrc, rsc, None,
                                op0=mybir.AluOpType.mult)
                        else:
                            nc.scalar.activation(
                                odst, psrc,
                                mybir.ActivationFunctionType.Identity,
                                scale=rsc)
                if variant == "pe":
                    continue
                nc.scalar.dma_start(
                    AP(outb, R * M * NB * C,
                       [[NB * C, M], [1, NBH * C]]),
                    ot[:, 0:NBH * C])
                nc.scalar.dma_start(
                    AP(outb, R * M * NB * C + NBH * C,
                       [[NB * C, M], [1, (NB - NBH) * C]]),
                    ot[:, NBH * C:])
    nc.compile()
    return nc


def _get_nc(rep=1, variant="full"):
    key = ("nc", rep, variant)
    if key not in _CACHE:
        _CACHE[key] = _build_nc(rep, variant)
    return _CACHE[key]


def _host_prep(x0, weights, cnts):
    # Padded global image: row g -> Xpad[g+3], col c -> +3, 129 channels.
    Xpad = np.zeros((H + 8, W + 21, CCH), BF16)
    Xpad[3:3 + H, 3:3 + W, :C] = x0.astype(BF16)
    Xpad[3:3 + H, 3:3 + W, C] = cnts[:, :, 0].astype(BF16)

    wq = weights.reshape(H, W, ROI, ROI)
    wpad = np.zeros((H, W + 13, ROI, ROI), np.float32)
    wpad[:, :W] = wq

    # band index grids: [R, rt, u, s, jj, t, b]
    Rg = np.arange(RG).reshape(-1, 1, 1, 1, 1, 1, 1)
    rt = np.arange(S).reshape(1, -1, 1, 1, 1, 1, 1)
    ug = np.arange(16).reshape(1, 1, -1, 1, 1, 1, 1)
    sg = np.arange(S).reshape(1, 1, 1, -1, 1, 1, 1)
    jj = np.arange(BLK).reshape(1, 1, 1, 1, -1, 1, 1)
    tg = np.arange(NT).reshape(1, 1, 1, 1, 1, -1, 1)
    bg = np.arange(NB).reshape(1, 1, 1, 1, 1, 1, -1)
    p = 8 * tg + rt - sg
    q = ug - jj
    cols = BLK * bg + jj
    mask = (p >= 0) & (p < ROI) & (q >= 0) & (q < ROI) & (cols < W)
    pc = np.clip(p, 0, ROI - 1)
    qc = np.clip(q, 0, ROI - 1)

    in_maps = []
    sr, sc, sch = Xpad.strides
    for k in range(NCORES):
        R0 = k * ROWS
        rows = R0 + S * Rg + sg
        val = wpad[rows, cols, pc, qc] * mask
        # val dims [R, rt, u, s, jj, t, b] -> [R, 128=(rt,u), t, s, jj, b]
        vt = val.transpose(0, 1, 2, 5, 3, 4, 6).reshape(
            RG, 128, NT, S, BLK, NB).astype(BF16)
        # zero-skip flat pack: per R, concat nonzero partition slabs
        bndarr = np.empty((RG, BNDSZ), BF16)
        for (t, s, p0, nparts, off) in _SLABS:
            bndarr[:, off:off + nparts * BLK * NB] = vt[
                :, p0:p0 + nparts, t, s].reshape(RG, -1)
        xbase = Xpad[R0:R0 + 76]
        xv = np.lib.stride_tricks.as_strided(
            xbase, shape=(NQ, S, 16, NB, CCH),
            strides=(S * sr, sr, sc, BLK * sc, sch))
        xqarr = np.ascontiguousarray(xv.reshape(NQ, 128, XFREE))
        in_maps.append({"xq": xqarr, "bnd": bndarr})
    return in_maps


def _unpack(outb):
    """outb [RG, 72, 57*128] -> [64, 512, 128] f32."""
    o = outb.astype(np.float32).reshape(RG, S, BLK, NB, C)
    o = o.transpose(0, 1, 3, 2, 4).reshape(ROWS, NB * BLK, C)
    return o[:, :W]


def kernel(x0, weights, cnts):
    x0 = np.asarray(x0, np.float32)
    weights = np.asarray(weights, np.float32)
    cnts = np.asarray(cnts, np.float32)
    nc = _get_nc()
    in_maps = _host_prep(x0, weights, cnts)
    res = run_bass_kernel_spmd(nc, in_maps, core_ids=list(range(NCORES)))
    return np.ascontiguousarray(np.concatenate(
        [_unpack(res.results[k]["outb"]) for k in range(NCORES)], axis=0))


# revision 22
# speedup vs baseline: 1.8521x; 1.0628x over previous
"""Trainium2 Bass kernel for nn_CellAnnotator (per-pixel 8x8 locally-connected
weighted pooling with normalization), SPMD across 8 NeuronCores.

Contract: kernel(**inputs) takes FULL inputs (x0 [512,512,128] f32,
weights [512,512,64] f32, cnts [512,512,1] f32) and returns the FULL
output [512,512,128] f32.

Sharding: rows (H) split across 8 cores, 64 output rows each; input shards
carry the 3+4-row halo host-side (zero-padded at borders) -> no d2d comms.

Algorithm (oct-row banded matmul, v2):
  Output rows in groups of S=8 (8 rowgroups/core); pixels in 57 blocks of 9
  (input window = 16 cols).  cnts ride as channel 128 of the x tiles, so ONE
  matmul per (rowgroup, block, oct) computes both pooled x and pooled cnts:
  lhsT = banded weights [K=128 (8 rows x 16 cols), M=72 (8 rows x 9 px)],
  rhs = x oct tile slice [128, 129]; 2 octs accumulate in PSUM.  Blocks are
  packed 3 per PSUM bank ([72, 387] f32), so the cnt columns of a chunk sit
  at stride 129 and one strided DVE reciprocal serves 3 blocks; the
  normalizing multiply alternates vector/scalar engines.

  Band tiles are stored zero-skip: layout [128, t, s, jj, b]; for column
  (s,jj) the oct-t band is nonzero only at partitions rt>=s (t=0) / rt<s
  (t=1), so HBM holds just those slabs (2.1x inflation instead of 4.2x) and
  the structural zeros are memset once into two fixed SBUF buffers.

  Traffic per core: x octs 16.9MB, banded weights 8.4MB, output 8.6MB.
"""

import numpy as np
import ml_dtypes
from contextlib import ExitStack

import concourse.bass as bass
import concourse.bacc as bacc
import concourse.mybir as mybir
import concourse.tile as tile
from concourse.ap import AP
from concourse.bass_utils import run_bass_kernel_spmd

BF16 = np.dtype(ml_dtypes.bfloat16)

# Problem constants (hardcoded per contract)
H, W, C = 512, 512, 128
ROI = 8
NCORES = 8
ROWS = H // NCORES             # 64 output rows per core
CCH = C + 1                    # x channels + cnts as channel 128

S = 8                          # output rows per rowgroup
RG = ROWS // S                 # 8 rowgroups per core
BLK = 9                        # output pixels per column block
NB = 57                        # column blocks (57*9 = 513 >= 512)
M = S * BLK                    # 72 live matmul output partitions
MPAD = 128                     # stationary padded to 128 cols -> FWL kicks in
NQ = 9                         # oct tiles per core (input rows 0..71)
NT = 2                         # octs (accumulation steps) per rowgroup
CH = 3                         # blocks per PSUM bank chunk (3*129 <= 512 f32)
NCH = NB // CH                 # 19 chunks per rowgroup
XFREE = NB * CCH               # 7353 elems per x oct partition

# zero-skip band storage, triangular by 16-partition bands: for column
# (s,jj) the oct-t band is nonzero at partitions rt>=s (t=0) / rt<s (t=1),
# i.e. partition band k (parts 16k..16k+16) holds s<=k for t=0 and s>k for
# t=1.  One DMA per (t, k) moves a [16, ns*513] block with contiguous
# per-partition runs (up to 8.2KB), keeping descriptor count low.
_SLABS = []          # (t, k, s0, ns, elem_offset)
_off = 0
for _t in range(NT):
    for _k in range(S):
        _s0, _ns = (0, _k + 1) if _t == 0 else (_k + 1, 7 - _k)
        if _ns <= 0:
            continue
        _SLABS.append((_t, _k, _s0, _ns, _off))
        _off += 16 * _ns * BLK * NB
BNDSZ = _off         # 525312 elems per rowgroup

_CACHE = {}


def _build_nc(rep=1, variant="full"):
    f32 = mybir.dt.float32
    bf = mybir.dt.bfloat16
    nc = bacc.Bacc("TRN2", target_bir_lowering=False, debug=False,
                   num_devices=NCORES)
    xq = nc.dram_tensor("xq", [NQ, 128, XFREE], bf, kind="ExternalInput")
    bnd = nc.dram_tensor("bnd", [RG, BNDSZ], bf, kind="ExternalInput")
    outb = nc.dram_tensor("outb", [RG, M, NB * C], bf, kind="ExternalOutput")

    with tile.TileContext(nc) as tc:
        with ExitStack() as ctx:
            xpool = ctx.enter_context(tc.tile_pool(name="xp", bufs=4))
            bpool = ctx.enter_context(tc.tile_pool(name="bp", bufs=2))
            ppool = ctx.enter_context(
                tc.tile_pool(name="pp", bufs=8, space="PSUM"))
            opool = ctx.enter_context(tc.tile_pool(name="op", bufs=3))
            spool = ctx.enter_context(tc.tile_pool(name="sp", bufs=3))

            # two fixed band buffers, structural zeros established once
            # (outside the rep loop); per-R DMAs only overwrite the
            # nonzero slabs, which are identical regions every iteration.
            # M dim padded to 128 zero columns so LDWEIGHTS gets FWL.
            bts = [bpool.tile([128, NT, MPAD, NB], bf, tag="bt",
                              name=f"bt{i}")
                   for i in range(2)]
            nc.vector.memset(bts[0][:], 0.0)
            nc.gpsimd.memset(bts[1][:], 0.0)

            def load_bands(R):
                bt = bts[R % 2]
                for i, (t, k, s0, ns, off) in enumerate(_SLABS):
                    eng = nc.gpsimd if i % 2 == 0 else nc.sync
                    eng.dma_start(
                        bt[16 * k:16 * k + 16, t,
                           BLK * s0:BLK * (s0 + ns), :],
                        AP(bnd, R * BNDSZ + off,
                           [[ns * BLK * NB, 16], [1, ns * BLK * NB]]))

            # rowgroup 0's bands load once before the rep loop; inside the
            # loop each iteration prefetches the NEXT rowgroup's bands
            # (wrapping to 0 for the next rep) before compute/out dispatch,
            # so no dispatch sits behind a blocking semaphore wait.
            load_bands(0)

            if rep > 1:
                ctx.enter_context(tc.For_i(0, rep, 1))

            xcache = {}
            HX = (NB // 2 + 1) * CCH

            def get_x(k):
                if k not in xcache:
                    # halves: block-b matmuls only read [b*CCH:...], so the
                    # first half-pass can start while half 2 is in flight
                    t = xpool.tile([128, XFREE], bf, tag="xt")
                    nc.sync.dma_start(
                        t[:, 0:HX], AP(xq, k * 128 * XFREE,
                                       [[XFREE, 128], [1, HX]]))
                    nc.sync.dma_start(
                        t[:, HX:], AP(xq, k * 128 * XFREE + HX,
                                      [[XFREE, 128], [1, XFREE - HX]]))
                    xcache[k] = t
                return xcache[k]

            for R in range(RG):
                bt = bts[R % 2]
                load_bands((R + 1) % RG)
                if variant == "dmab":
                    continue
                xts = [get_x(R + t) for t in range(NT)]
                if variant == "dma":
                    continue

                rec = spool.tile([M, NB], f32, tag="rec")
                ot = opool.tile([M, NB * C], bf, tag="ot")
                NBH = NB // 2 + 1
                for c in range(NCH):
                    psum = ppool.tile([MPAD, CH * CCH], f32, tag="ps")
                    for j in range(CH):
                        b = c * CH + j
                        for t in range(NT):
                            nc.tensor.matmul(
                                psum[:, j * CCH:(j + 1) * CCH],
                                bt[:, t, :, b],
                                AP(xts[t][:].tensor, b * CCH,
                                   [[XFREE, 128], [1, CCH]]),
                                start=(t == 0), stop=(t == NT - 1))
                    if variant == "pe":
                        continue
                    # batched reciprocal of the 3 cnt columns (stride 129)
                    nc.vector.reciprocal(
                        rec[:, c * CH:(c + 1) * CH],
                        AP(psum[:].tensor, C, [[CH * CCH, M], [CCH, CH]]))
                    for j in range(CH):
                        b = c * CH + j
                        odst = ot[:, b * C:(b + 1) * C]
                        psrc = AP(psum[:].tensor, j * CCH,
                                  [[CH * CCH, M], [1, C]])
                        rsc = rec[:, c * CH + j:c * CH + j + 1]
                        # NOTE: GPSIMD cannot access PSUM on HW (sim allows)
                        if b % 2 == 0:
                            nc.vector.tensor_scalar(
                                odst, psrc, rsc, None,
                                op0=mybir.AluOpType.mult)
                        else:
                            nc.scalar.activation(
                                odst, psrc,
                                mybir.ActivationFunctionType.Identity,
                                scale=rsc)
                if variant == "pe":
                    continue
                nc.gpsimd.dma_start(
                    AP(outb, R * M * NB * C,
                       [[NB * C, M], [1, NBH * C]]),
                    ot[:, 0:NBH * C])
                nc.gpsimd.dma_start(
                    AP(outb, R * M * NB * C + NBH * C,
                       [[NB * C, M], [1, (NB - NBH) * C]]),
                    ot[:, NBH * C:])
    nc.compile()
    return nc


def _get_nc(rep=1, variant="full"):
    key = ("nc", rep, variant)
    if key not in _CACHE:
        _CACHE[key] = _build_nc(rep, variant)
    return _CACHE[key]


def _host_prep(x0, weights, cnts):
    # Padded global image: row g -> Xpad[g+3], col c -> +3, 129 channels.
    Xpad = np.zeros((H + 8, W + 21, CCH), BF16)
    Xpad[3:3 + H, 3:3 + W, :C] = x0.astype(BF16)
    Xpad[3:3 + H, 3:3 + W, C] = cnts[:, :, 0].astype(BF16)

    wq = weights.reshape(H, W, ROI, ROI)
    wpad = np.zeros((H, W + 13, ROI, ROI), np.float32)
    wpad[:, :W] = wq

    # band index grids: [R, rt, u, s, jj, t, b]
    Rg = np.arange(RG).reshape(-1, 1, 1, 1, 1, 1, 1)
    rt = np.arange(S).reshape(1, -1, 1, 1, 1, 1, 1)
    ug = np.arange(16).reshape(1, 1, -1, 1, 1, 1, 1)
    sg = np.arange(S).reshape(1, 1, 1, -1, 1, 1, 1)
    jj = np.arange(BLK).reshape(1, 1, 1, 1, -1, 1, 1)
    tg = np.arange(NT).reshape(1, 1, 1, 1, 1, -1, 1)
    bg = np.arange(NB).reshape(1, 1, 1, 1, 1, 1, -1)
    p = 8 * tg + rt - sg
    q = ug - jj
    cols = BLK * bg + jj
    mask = (p >= 0) & (p < ROI) & (q >= 0) & (q < ROI) & (cols < W)
    pc = np.clip(p, 0, ROI - 1)
    qc = np.clip(q, 0, ROI - 1)

    in_maps = []
    sr, sc, sch = Xpad.strides
    for k in range(NCORES):
        R0 = k * ROWS
        rows = R0 + S * Rg + sg
        val = wpad[rows, cols, pc, qc] * mask
        # val dims [R, rt, u, s, jj, t, b] -> [R, 128=(rt,u), t, m=(s,jj), b]
        vt = val.transpose(0, 1, 2, 5, 3, 4, 6).reshape(
            RG, 128, NT, M, NB).astype(BF16)
        # zero-skip flat pack: per R, concat triangular (t, k-band) blocks
        bndarr = np.empty((RG, BNDSZ), BF16)
        for (t, k, s0, ns, off) in _SLABS:
            bndarr[:, off:off + 16 * ns * BLK * NB] = vt[
                :, 16 * k:16 * k + 16, t,
                BLK * s0:BLK * (s0 + ns)].reshape(RG, -1)
        xbase = Xpad[R0:R0 + 76]
        xv = np.lib.stride_tricks.as_strided(
            xbase, shape=(NQ, S, 16, NB, CCH),
            strides=(S * sr, sr, sc, BLK * sc, sch))
        xqarr = np.ascontiguousarray(xv.reshape(NQ, 128, XFREE))
        in_maps.append({"xq": xqarr, "bnd": bndarr})
    return in_maps


def _unpack(outb):
    """outb [RG, 72, 57*128] -> [64, 512, 128] f32."""
    o = outb.astype(np.float32).reshape(RG, S, BLK, NB, C)
    o = o.transpose(0, 1, 3, 2, 4).reshape(ROWS, NB * BLK, C)
    return o[:, :W]


def kernel(x0, weights, cnts):
    x0 = np.asarray(x0, np.float32)
    weights = np.asarray(weights, np.float32)
    cnts = np.asarray(cnts, np.float32)
    nc = _get_nc()
    in_maps = _host_prep(x0, weights, cnts)
    res = run_bass_kernel_spmd(nc, in_maps, core_ids=list(range(NCORES)))
    return np.ascontiguousarray(np.concatenate(
        [_unpack(res.results[k]["outb"]) for k in range(NCORES)], axis=0))
